# revision 76
# baseline (speedup 1.0000x reference)
"""Trainium2 Bass kernel for nn_Mhsa_47802986004933.

Model (per batch b of 2):
  BN(train-stats)+ReLU -> 1x1 conv qkv (raw .view reinterpret) ->
  4-head attention on heads 0-3  +  conv-mixing (3x1 / 1x3) on heads 4-7 ->
  concat -> kernel-2 avg pool.

Sharding: 8 cores = (batch b in {0,1}) x (h in {0..3}).
  Core c = 4b + h:
    - full 4096x4096 attention for head h of batch b  -> out[b, :, 32h:32h+32]
    - conv y-quarter [16h, 16h+16)                    -> out[b, n%16 in [4h,4h+4), 128:256]
  Communication-free SPMD: BN stats recomputed on every core from the full x.

Key structural identity: with O = W @ xn [1536, 4096] per batch and
U = O.reshape(12288, 512) (u = 8o+g), token n has q = U[3n], k = U[3n+1],
v = U[3n+2].  Attention head h uses U columns [64h, 64h+64); the conv branch
uses columns [256, 512) with image layout q2[i, y, x] =
U[3*(64*(i%64)+y), 256 + 64*(i//64) + x].

Attention engine split: PE does QK^T (512-col tiles) and a transposed AV
(out[q,33] = exp-scores^T @ [Vavg|1]) with V pre-pair-averaged (the trailing
avg-pool commutes with attention).  Softmax exp is split across ACT and DVE,
both computing the identical Schraudolph fast-exp int16(x*A+B) bitcast to
bf16 (~3% max rel err, bias cancels in the softmax normalization; Pool is
barred from PSUM by the BIR verifier so only two engines can exp).
"""
import os
import sys
import numpy as np
import ml_dtypes

sys.path.insert(0, "/opt/trn_rl_repo")

import concourse.bass as bass
import concourse.bacc as bacc
import concourse.mybir as mybir
import concourse.tile as tile
from concourse import bass_utils

B, N, DIM, S = 2, 4096, 256, 64
H, DH, INNER = 8, 64, 512
EPS = 1e-5
FP = mybir.dt.float32
FR = mybir.dt.float32r
BF = mybir.dt.bfloat16
I32 = mybir.dt.int32
I16 = mybir.dt.int16
I8 = mybir.dt.int8
F8 = mybir.dt.float8e5
AF = mybir.ActivationFunctionType
OP = mybir.AluOpType

NG = 16            # score groups per 512-query chunk (2 key-blocks each)
LAG = 3            # AV trails QK by this many groups
# exp engine per group: A=ACT, D=DVE (Pool cannot read PSUM on real TRN2)
EXPENG = "ADADADADADADADAA"
assert len(EXPENG) == NG
# Schraudolph fast-exp of (0.125 * x) in bf16: int16 bits = x*EXPA + EXPB,
# read as bf16 (bf16 matmuls dodge the FP32r-rounding verifier rule)
EXPA = 0.125 * (1 << 7) * 1.4426950408889634
EXPB = float(127 << 7) - 366393.0 / 65536.0 + 0.5


def _r(ap):
    return ap.bitcast(FR)


def build_device_program():
    nc = bacc.Bacc(
        "TRN2", target_bir_lowering=False, debug=False, enable_asserts=True,
        num_devices=8,
    )
    xts = nc.dram_tensor("xts", [256, 8192], BF, kind="ExternalInput").ap()
    xc_d = nc.dram_tensor("xc", [256, 2560], BF, kind="ExternalInput").ap()
    wq_d = nc.dram_tensor("wq", [256, 1536], BF, kind="ExternalInput").ap()
    wcg_d = nc.dram_tensor("wcg", [256, 3072], BF, kind="ExternalInput").ap()
    wch_d = nc.dram_tensor("wch", [256, 128], BF, kind="ExternalInput").ap()
    w1s_d = nc.dram_tensor("w1s", [256, 768], FP, kind="ExternalInput").ap()
    w2s_d = nc.dram_tensor("w2s", [256, 768], FP, kind="ExternalInput").ap()
    gb_d = nc.dram_tensor("gb", [256, 2], FP, kind="ExternalInput").ap()
    pairm_d = nc.dram_tensor("pairm", [64, 32], FP, kind="ExternalInput").ap()
    out_a = nc.dram_tensor("out_a", [4096, 32], FP, kind="ExternalOutput").ap()
    out_c = nc.dram_tensor("out_c", [1024, 128], FP, kind="ExternalOutput").ap()

    with tile.TileContext(nc) as tc:
        _emit(tc, nc, xts, xc_d, wq_d, wcg_d, wch_d, w1s_d, w2s_d, gb_d,
              pairm_d, out_a, out_c)
    nc.compile()
    return nc


def _emit(tc, nc, xts, xc_d, wq_d, wcg_d, wch_d, w1s_d, w2s_d, gb_d,
          pairm_d, out_a, out_c):
    from contextlib import ExitStack
    ctx = ExitStack()
    with ctx:
        cp = ctx.enter_context(tc.tile_pool(name="const", bufs=1))
        sctx = ExitStack()
        sp = sctx.enter_context(tc.tile_pool(name="scratch", bufs=1))
        xctx = ExitStack()
        xp = xctx.enter_context(tc.tile_pool(name="xload", bufs=1))
        pctx = ExitStack()
        pm = pctx.enter_context(tc.tile_pool(name="ps_m", bufs=6, space="PSUM"))

        dma = nc.sync.dma_start
        vec = nc.vector
        act = nc.scalar
        gp = nc.gpsimd

        # ---------------- persistent SBUF ----------------
        ut = cp.tile([64, 12288], FR, tag="ut", name="ut")
        vsb = cp.tile([128, 1056], BF, tag="vsb", name="vsb")
        psbs = [cp.tile([128, 1024], BF, tag=f"psb{k}", name=f"psb{k}")
                for k in range(LAG + 1)]
        resbs = [cp.tile([128, 128], FP, tag=f"resb{k}", name=f"resb{k}")
                 for k in range(2)]
        rec4s = [cp.tile([128, 4], FP, tag=f"rec4{k}", name=f"rec4{k}")
                 for k in range(2)]
        onec = cp.tile([128, 1], FP, tag="onec", name="onec")
        vec.memset(onec, 1.0)
        zconst = cp.tile([128, 1], FP, tag="zconst", name="zconst")
        vec.memset(zconst, 0.0)
        nc.const_aps.aps[(FP, 0.0)] = zconst

        # scratch-lifetime tiles (front phase; bf16 weights halve the DMA
        # on the critical path -- proj matmuls are all-bf16, psum is fp32)
        wq = [sp.tile([128, 1536], BF, tag="wq0", name="wq0"),
              sp.tile([128, 1536], BF, tag="wq1", name="wq1")]
        w1s = [cp.tile([128, 768], FR, tag="w1s0", name="w1s0"),
               cp.tile([128, 768], FR, tag="w1s1", name="w1s1")]
        w2s = [cp.tile([128, 768], FR, tag="w2s0", name="w2s0"),
               cp.tile([128, 768], FR, tag="w2s1", name="w2s1")]
        gb = [sp.tile([128, 2], FP, tag="gb0", name="gb0"),
              sp.tile([128, 2], FP, tag="gb1", name="gb1")]
        pairm = sp.tile([64, 32], FR, tag="pairm", name="pairm")
        wch = [sp.tile([128, 128], BF, tag="wch0", name="wch0"),
               sp.tile([128, 128], BF, tag="wch1", name="wch1")]
        xn = [sp.tile([128, 2560], BF, tag="xn0", name="xn0"),
              sp.tile([128, 2560], BF, tag="xn1", name="xn1")]
        # permute-DMA sources live in the persistent pool so that closing
        # the scratch pool does not barrier on those (long) DMAs
        sq2 = cp.tile([128, 2048], FR, tag="sq2", name="sq2")
        sk2 = cp.tile([128, 2048], FR, tag="sk2", name="sk2")
        sv2 = cp.tile([128, 2048], FR, tag="sv2", name="sv2")
        qhalo = [cp.tile([64, 256], FR, tag="qhalo0", name="qhalo0"),
                 cp.tile([64, 256], FR, tag="qhalo1", name="qhalo1")]
        q2q = [cp.tile([128, 1152], FR, tag="q2q0", name="q2q0"),
               cp.tile([128, 1152], FR, tag="q2q1", name="q2q1")]
        k2q = [cp.tile([128, 1024], FR, tag="k2q0", name="k2q0"),
               cp.tile([128, 1024], FR, tag="k2q1", name="k2q1")]
        v2q = [cp.tile([128, 1024], FR, tag="v2q0", name="v2q0"),
               cp.tile([128, 1024], FR, tag="v2q1", name="v2q1")]

        # ---------------- x load + BN stats (DVE) ----------------
        # x tiles first (stats are the critical chain; the first tile is
        # split into 512-col pieces so bn_stats starts ASAP); per-hf affine
        # emitted right after that hf's stats so DVE computes it before
        # grinding through the other half's stats.
        xcb = [sp.tile([128, 2560], BF, tag=f"xcb{hf}", name=f"xcb{hf}")
               for hf in range(2)]
        bnst = [xp.tile([128, 96], FP, tag=f"bnst{hf}", name=f"bnst{hf}")
                for hf in range(2)]
        epst = cp.tile([128, 1], FP, tag="epst", name="epst")
        vec.memset(epst, EPS)
        aff = []
        for hf in range(2):
            for j in range(4):
                xkt = xp.tile([128, 2048], BF, tag="xk", bufs=3,
                              name=f"xk{hf}{j}")
                if hf == 0 and j == 0:
                    for sub in range(4):
                        dma(out=xkt[:, 512 * sub:512 * sub + 512],
                            in_=xts[0:128, 512 * sub:512 * sub + 512])
                else:
                    dma(out=xkt, in_=xts[128 * hf:128 * hf + 128,
                                         2048 * j:2048 * j + 2048])
                for sub in range(4):
                    kk = 4 * j + sub
                    vec.bn_stats(out=bnst[hf][:, 6 * kk:6 * kk + 6],
                                 in_=xkt[:, 512 * sub:512 * sub + 512])
            if hf == 0:
                for h2 in range(2):
                    dma(out=gb[h2], in_=gb_d[128 * h2:128 * h2 + 128, :])
            mv = xp.tile([128, 2], FP, tag=f"mv{hf}", name=f"mv{hf}")
            vec.bn_aggr(out=mv, in_=bnst[hf].rearrange("p (k s) -> p k s", s=6))
            sqv = xp.tile([128, 1], FP, tag=f"sqv{hf}", name=f"sqv{hf}")
            act.activation(sqv, mv[:, 1:2], AF.Sqrt, bias=epst)
            rsv = xp.tile([128, 1], FP, tag=f"rsv{hf}", name=f"rsv{hf}")
            vec.reciprocal(rsv, sqv)
            a_ = xp.tile([128, 1], FP, tag=f"a{hf}", name=f"a{hf}")
            vec.tensor_tensor(a_, rsv, gb[hf][:, 0:1], OP.mult)
            tmp = xp.tile([128, 1], FP, tag=f"tmp{hf}", name=f"tmp{hf}")
            vec.tensor_tensor(tmp, mv[:, 0:1], a_, OP.mult)
            bb = xp.tile([128, 1], FP, tag=f"bb{hf}", name=f"bb{hf}")
            vec.tensor_tensor(bb, gb[hf][:, 1:2], tmp, OP.subtract)
            aff.append((a_, bb))

        # ---------------- weight loads ----------------
        for hf in range(2):
            dma(out=xcb[hf], in_=xc_d[128 * hf:128 * hf + 128, :])
        for hf in range(2):
            dma(out=wq[hf], in_=wq_d[128 * hf:128 * hf + 128, :])
        dma(out=pairm, in_=pairm_d.bitcast(FR))
        for hf in range(2):
            dma(out=w1s[hf], in_=w1s_d.bitcast(FR)[128 * hf:128 * hf + 128, :])
            dma(out=w2s[hf], in_=w2s_d.bitcast(FR)[128 * hf:128 * hf + 128, :])
            dma(out=wch[hf], in_=wch_d[128 * hf:128 * hf + 128, :])

        # ---------------- xn = relu(a*x+b) ----------------
        for hf in range(2):
            a_, bb = aff[hf]
            act.activation(xn[hf], xcb[hf], AF.Relu, bias=bb, scale=a_)
        xctx.close()

        # ---------------- head projection -> ut ----------------
        # paired groups: ps[0:64]=group 2gp, ps[64:128]=group 2gp+1
        # (PSUM readers can only be DVE/ACT; Pool is PSUM-banned.)
        # During proj DVE still drains bn_stats, so ACT takes 2/3 of the
        # copies; later phases rebalance.
        cyc = [vec, act]
        ci = 0
        for gpi in range(4):
            for oc in range(3):
                ps = pm.tile([128, 512], FP, tag="m", name=f"pr{gpi}_{oc}")
                nc.tensor.matmul(ps, xn[0][:, 128 * gpi:128 * gpi + 128],
                                 wq[0][:, 512 * oc:512 * oc + 512],
                                 start=True, stop=False)
                nc.tensor.matmul(ps, xn[1][:, 128 * gpi:128 * gpi + 128],
                                 wq[1][:, 512 * oc:512 * oc + 512],
                                 start=False, stop=True)
                for half in range(2):
                    g = 2 * gpi + half
                    dst = ut[:, 4096 * oc + g: 4096 * oc + g + 4089: 8]
                    src = ps[64 * half:64 * half + 64, :]
                    eng = cyc[ci % 2]
                    ci += 1
                    if eng is act:
                        act.activation(dst, src, AF.Copy)
                    else:
                        eng.tensor_copy(dst, src)

        # halo rows (j=0): lo rho=7 g=5 ; hi rho=0 g=0
        for e, wcol, gg in ((0, 0, 5), (1, 64, 0)):
            ph = pm.tile([64, 256], FP, tag="m", name=f"phalo{e}")
            nc.tensor.matmul(ph, wch[0][:, wcol:wcol + 64],
                             xn[0][:, 512 + 256 * gg:512 + 256 * gg + 256],
                             start=True, stop=False)
            nc.tensor.matmul(ph, wch[1][:, wcol:wcol + 64],
                             xn[1][:, 512 + 256 * gg:512 + 256 * gg + 256],
                             start=False, stop=True)
            vec.tensor_copy(qhalo[e], ph)

        # ------- conv-input slim projection + per-j permute DMAs --------
        # the permute DMAs fire as each slim tensor completes, so they
        # overlap the remaining projections and the attention phase
        def permute_dmas(srct, dstt, off):
            for ci2 in range(2):
                for hh in range(2):
                    for ya in range(2):
                        src = srct[64 * ya:64 * ya + 64, :].rearrange(
                            "i (r h x) -> h i r x", r=8, h=4, x=64)[2 * ci2 + hh]
                        dst = dstt[ci2][64 * hh:64 * hh + 64,
                                        off + 512 * ya:off + 512 * ya + 512
                                        ].rearrange("i (r x) -> i r x", x=64)
                        dma(out=dst, in_=src)

        for j, dst, dstt, off in ((0, sq2, q2q, 64), (1, sk2, k2q, 0),
                                  (2, sv2, v2q, 0)):
            wcgj = [sp.tile([128, 1024], BF, tag="wcgj0", bufs=2,
                            name=f"wcgj0_{j}"),
                    sp.tile([128, 1024], BF, tag="wcgj1", bufs=2,
                            name=f"wcgj1_{j}")]
            for hf in range(2):
                dma(out=wcgj[hf], in_=wcg_d[
                    128 * hf:128 * hf + 128, 1024 * j:1024 * j + 1024])
            for rho in range(8):
                g = (3 * rho + j) % 8
                ps = pm.tile([128, 256], FP, tag="m", name=f"pc{j}_{rho}")
                nc.tensor.matmul(ps, wcgj[0][:, 128 * rho:128 * rho + 128],
                                 xn[0][:, 512 + 256 * g:512 + 256 * g + 256],
                                 start=True, stop=False)
                nc.tensor.matmul(ps, wcgj[1][:, 128 * rho:128 * rho + 128],
                                 xn[1][:, 512 + 256 * g:512 + 256 * g + 256],
                                 start=False, stop=True)
                eng = cyc[ci % 2]
                ci += 1
                if eng is act:
                    act.activation(dst[:, 256 * rho:256 * rho + 256], ps,
                                   AF.Copy)
                else:
                    eng.tensor_copy(dst[:, 256 * rho:256 * rho + 256], ps)

        # ---------------- V tiles: pair-sum via PE matmul + ones --------
        # vsb[key, e] = V[key, 2e] + V[key, 2e+1]:
        # matmul(out[128 keys, 32], lhsT=ut_v[64 d, 128 keys], rhs=pairm)
        cyc2 = cyc
        vec.tensor_copy(vsb[:, 32::33], onec.to_broadcast((128, 32)))
        for t in range(32):
            pv = pm.tile([128, 32], FP, tag="m", name=f"vt{t}")
            nc.tensor.matmul(pv, ut[:, 3 * 128 * t + 2: 3 * 128 * t + 384: 3],
                             pairm, start=True, stop=True)
            eng = cyc[ci % 2]
            ci += 1
            if eng is act:
                act.activation(vsb[:, 33 * t:33 * t + 32], pv, AF.Copy)
            else:
                eng.tensor_copy(vsb[:, 33 * t:33 * t + 32], pv)

        # release scratch + front psum; open late SBUF and attention PSUM
        sctx.close()
        pctx.close()
        kp = ctx.enter_context(tc.tile_pool(name="late", bufs=1))
        wp = ctx.enter_context(tc.tile_pool(name="work", bufs=2))
        spools = [ctx.enter_context(
            tc.tile_pool(name=f"ps_s{k}", bufs=1, space="PSUM"))
            for k in range(3)]
        po = ctx.enter_context(tc.tile_pool(name="ps_o", bufs=2, space="PSUM"))

        k2l = [kp.tile([128, 1024], FR, tag="k2l0", name="k2l0"),
               kp.tile([128, 1024], FR, tag="k2l1", name="k2l1")]
        k2r = [kp.tile([128, 1024], FR, tag="k2r0", name="k2r0"),
               kp.tile([128, 1024], FR, tag="k2r1", name="k2r1")]
        v2p = [kp.tile([128, 512], FP, tag="v2p0", name="v2p0"),
               kp.tile([128, 512], FP, tag="v2p1", name="v2p1")]
        pavb = [kp.tile([128, 512], FP, tag="pavb0", name="pavb0"),
                kp.tile([128, 512], FP, tag="pavb1", name="pavb1")]

        # permute DMAs, emitted after the pool transition so the close-drain
        # does not wait on them; they overlap the attention phase
        for _, srct, dstt, off in ((0, sq2, q2q, 64), (1, sk2, k2q, 0),
                                   (2, sv2, v2q, 0)):
            permute_dmas(srct, dstt, off)
        for ci2 in range(2):
            for hh in range(2):
                for e, dlo, dhi in ((0, 0, 64), (1, 1088, 1152)):
                    src = qhalo[e].rearrange(
                        "i (h x) -> h i x", h=4)[2 * ci2 + hh]
                    dma(out=q2q[ci2][64 * hh:64 * hh + 64, dlo:dhi],
                        in_=src)

        # k2 shifted-by-x copies + v2 pair-sums on Pool (SBUF-only work,
        # overlaps the attention phase on an otherwise idle engine)
        for ci2 in range(2):
            kv = k2q[ci2].rearrange("p (y x) -> p y x", x=64)
            gp.tensor_copy(k2l[ci2][:, 63::64], zconst.to_broadcast((128, 16)))
            lv = k2l[ci2].rearrange("p (y x) -> p y x", x=64)
            gp.tensor_copy(lv[:, :, 0:63], kv[:, :, 1:64])
            gp.tensor_copy(k2r[ci2][:, 0::64], zconst.to_broadcast((128, 16)))
            rv = k2r[ci2].rearrange("p (y x) -> p y x", x=64)
            gp.tensor_copy(rv[:, :, 1:64], kv[:, :, 0:63])
            vv = v2q[ci2].rearrange("p (e two) -> p e two", two=2)
            gp.tensor_tensor(v2p[ci2], vv[:, :, 0], vv[:, :, 1], OP.add)



        # ---------------- attention ----------------
        stages = [(c, g) for c in range(8) for g in range(NG)]
        pavs = {}

        def emit_qk_exp(i):
            c, g = stages[i]
            rhs_q = ut[:, 3 * 512 * c: 3 * 512 * c + 1534: 3]
            pss = spools[i % 3].tile([128, 1024], FP, tag=f"s{i % 3}",
                                     name=f"s{c}_{g}")
            for q in range(2):
                t = 2 * g + q
                nc.tensor.matmul(
                    pss[:, 512 * q:512 * q + 512],
                    ut[:, 3 * 128 * t + 1: 3 * 128 * t + 383: 3],
                    rhs_q, start=True, stop=True, skip_group_check=True)
            psb = psbs[i % (LAG + 1)]
            e = EXPENG[g]
            if e == "A":
                # identical bit-pattern to the DVE fast-exp path, so the
                # approximation bias cancels exactly in the softmax normalizer
                act.activation(psb.bitcast(I16), pss, AF.Copy,
                               bias=EXPB, scale=EXPA)
            else:
                vec.tensor_scalar(psb.bitcast(I16), pss, EXPA, EXPB,
                                  OP.mult, OP.add)

        def emit_av(i):
            c, g = stages[i]
            if g == 0:
                pavs[c] = po.tile([128, 132], FP, tag="o", name=f"o{c}")
            pav = pavs[c]
            psb = psbs[i % (LAG + 1)]
            for q in range(2):
                t = 2 * g + q
                for s in range(4):
                    # the whole 132-col tile lives in one PSUM bank: exactly
                    # one start=True (t=0, s=0) zeroes the bank; the other
                    # regions accumulate onto pending-zero bytes
                    nc.tensor.matmul(
                        pav[:, 33 * s:33 * s + 33],
                        psb[:, 512 * q + 128 * s:512 * q + 128 * s + 128],
                        vsb[:, 33 * t:33 * t + 33],
                        start=(t == 0 and s == 0), stop=(t == 31),
                        skip_group_check=True)
            if g == NG - 1:
                rec4 = rec4s[c % 2]
                vec.reciprocal(rec4, pav[:, 32::33])
                resb = resbs[c % 2]
                vec.scalar_tensor_tensor(
                    out=resb.rearrange("p (s e) -> p s e", s=4),
                    in0=pav.rearrange("p (s e) -> p s e", e=33)[:, :, 0:32],
                    scalar=0.5, in1=rec4.to_broadcast((128, 4, 32)),
                    op0=OP.mult, op1=OP.mult)
                dma(out=out_a[512 * c:512 * c + 512, :].rearrange(
                        "(s p) e -> p s e", s=4),
                    in_=resb.rearrange("p (s e) -> p s e", s=4))

        for i in range(len(stages)):
            emit_qk_exp(i)
            if i >= LAG:
                emit_av(i - LAG)
        for i in range(len(stages) - LAG, len(stages)):
            emit_av(i)

        # ---------------- conv matmuls + pair-avg + store ----------------
        # PSUM: reuse score-pool tiles; each (oc, ch) accumulation region is
        # one bank-aligned 512-col half of a [128, 1024] spool tile
        for oc in range(2):
            cv = spools[oc].tile([128, 1024], FP, tag=f"s{oc}",
                                 name=f"cv{oc}")
            for ch in range(2):
                ps = cv[:, 512 * ch:512 * ch + 512]
                k = 0
                for dy in range(3):
                    for hf in range(2):
                        nc.tensor.matmul(
                            ps, w1s[hf][:, 256 * dy + 128 * oc:256 * dy + 128 * oc + 128],
                            q2q[hf][:, 512 * ch + 64 * dy:512 * ch + 64 * dy + 512],
                            start=(k == 0), stop=False, skip_group_check=True)
                        k += 1
                for dx, srcb in ((0, k2r), (1, k2q), (2, k2l)):
                    for hf in range(2):
                        nc.tensor.matmul(
                            ps, w2s[hf][:, 256 * dx + 128 * oc:256 * dx + 128 * oc + 128],
                            srcb[hf][:, 512 * ch:512 * ch + 512],
                            start=False, stop=(k == 11), skip_group_check=True)
                        k += 1
                cop = wp.tile([128, 512], FP, tag="cop", name=f"cop{oc}{ch}")
                act.activation(cop, ps, AF.Copy)
                pav = pavb[oc][:, 256 * ch:256 * ch + 256]
                csv = cop.rearrange("p (e two) -> p e two", two=2)
                vec.tensor_tensor(pav, csv[:, :, 0], csv[:, :, 1], OP.add)
                vec.tensor_tensor(pav, pav,
                                  v2p[oc][:, 256 * ch:256 * ch + 256], OP.add)
            dma(out=out_c.rearrange("(o w) e -> o w e", w=4)[
                    128 * oc:128 * oc + 128, :, :],
                in_=pavb[oc].rearrange("p (w e) -> p w e", w=4))


# =====================================================================
# Host side
# =====================================================================
_NC_CACHE = None


def _get_nc():
    global _NC_CACHE
    if _NC_CACHE is None:
        _NC_CACHE = build_device_program()
    return _NC_CACHE


def make_in_maps(x, qkv_w, bn_gamma, bn_beta, conv1_w, conv2_w):
    x = np.asarray(x, np.float32)
    WT = np.ascontiguousarray(np.asarray(qkv_w, np.float32).T)   # [256, 1536]
    WTb = WT.astype(ml_dtypes.bfloat16)
    xT = np.ascontiguousarray(x.transpose(0, 2, 1))              # [2, 256, 4096]
    xts = np.ascontiguousarray(
        np.concatenate([xT[0], xT[1]], axis=1).astype(ml_dtypes.bfloat16))
    w1s = np.ascontiguousarray(
        0.5 * np.asarray(conv1_w, np.float32)[:, :, :, 0].transpose(1, 2, 0)
        .reshape(256, 768))                                      # [i, dy*256+o]
    w2s = np.ascontiguousarray(
        0.5 * np.asarray(conv2_w, np.float32)[:, :, 0, :].transpose(1, 2, 0)
        .reshape(256, 768))
    gbar = np.ascontiguousarray(
        np.stack([np.asarray(bn_gamma, np.float32),
                  np.asarray(bn_beta, np.float32)], axis=1))     # [256, 2]
    pairm = np.kron(np.eye(32, dtype=np.float32),
                    np.ones((2, 1), np.float32))                 # [64, 32]

    ilo = np.arange(64)
    in_maps = []
    for c in range(8):
        b, h = c // 4, c % 4
        head_cols = np.concatenate(
            [512 * g + 64 * h + np.arange(64) for g in range(8)])
        conv_cols = np.concatenate(
            [512 * g + 256 + np.arange(256) for g in range(8)])
        xc = np.ascontiguousarray(
            xT[b][:, np.concatenate([head_cols, conv_cols])]
            .astype(ml_dtypes.bfloat16))                         # [256, 2560]
        # slim conv-proj weights: col (j*8+rho)*128 + 64*mr + ilo
        #   -> WT col (3*rho+j)//8 + 3*(2h+mr) + 24*ilo   (j=2 scaled by 0.5)
        wcg = np.zeros((256, 3072), np.float32)
        for j in range(3):
            sc = 0.5 if j == 2 else 1.0
            for rho in range(8):
                o0 = (3 * rho + j) // 8
                for mr in range(2):
                    cols = o0 + 3 * (2 * h + mr) + 24 * ilo
                    wcg[:, (j * 8 + rho) * 128 + 64 * mr + ilo] = sc * WT[:, cols]
        # halo: lo (rho=7, ya=2h-1): o = 2 + 3*(2h-1) + 24*ilo   (h>=1)
        #       hi (rho=0, ya=2h+2): o = 3*(2h+2) + 24*ilo       (h<=2)
        wch = np.zeros((256, 128), np.float32)
        if h >= 1:
            wch[:, 0:64] = WT[:, 2 + 3 * (2 * h - 1) + 24 * ilo]
        if h <= 2:
            wch[:, 64:128] = WT[:, 3 * (2 * h + 2) + 24 * ilo]
        in_maps.append({
            "xts": xts, "xc": xc, "wq": WTb, "wcg": wcg.astype(ml_dtypes.bfloat16),
            "wch": wch.astype(ml_dtypes.bfloat16),
            "w1s": w1s, "w2s": w2s, "gb": gbar, "pairm": pairm,
        })
    return in_maps


def assemble(results):
    out = np.zeros((B, N, DIM), np.float32)
    for c in range(8):
        b, h = c // 4, c % 4
        out[b, :, 32 * h:32 * h + 32] = results[c]["out_a"]
        oc = results[c]["out_c"].reshape(256, 4, 128)
        out[b].reshape(256, 16, 256)[:, 4 * h:4 * h + 4, 128:256] = oc
    return out


def kernel(**inputs):
    nc = _get_nc()
    in_maps = make_in_maps(**inputs)
    res = bass_utils.run_bass_kernel_spmd(
        nc, in_maps, core_ids=list(range(8)),
        trace=bool(int(os.environ.get("KERNEL_TRACE", "0"))))
    out = assemble(res.results)
    if res.exec_time_ns is not None:
        print(f"HW exec time: {res.exec_time_ns} ns", file=sys.stderr)
        kernel.last_exec_time_ns = res.exec_time_ns
    kernel.last_results = res
    return out


kernel.last_exec_time_ns = None
kernel.last_results = None
